# revision 18
# baseline (speedup 1.0000x reference)
"""Trainium2 Bass kernel for nn_ComplexMixture: weighted complex Gram matrices.

Reference (per batch b, inputs real/imag [B,T,D] f32, weight [B,T,1] f32):
    out_r[b] = sum_t w[b,t] * (r_t r_t^T + i_t i_t^T)   (symmetric)
    out_i[b] = sum_t w[b,t] * (i_t r_t^T - r_t i_t^T)   (antisymmetric)
with B=64, T=256, D=512; outputs (out_r, out_i), each [B, D, D] f32.

Pure data-parallel over 8 NeuronCores (8 batches per core).

v2 design (vs v1 at ~143 us/iter):
  - Host pre-scales: a = fp16(sqrt(w)*r), c = fp16(sqrt(w)*i). Removes the
    on-device weight DMA (2048 4-byte descriptors!), the ACT scaling stage,
    and halves input DMA bytes (f32 -> fp16).
  - Host pre-transposes into IN[128, BPC*KT*2*D] (partition = time-within-
    K-tile) so input DMA is 4x 1MB chunks with 128x8KB fully contiguous
    descriptors (line rate ~341 GB/s), alternated across the two HWDGE
    rings (nc.sync / nc.scalar).
  - Diagonal-block combine: only M = D_r + D_i is stored for the 4 diagonal
    blocks (host recovers D_r=(M+M^T)/2, D_i=(M-M^T)/2 since D_r symmetric,
    D_i antisymmetric). Off-diag upper blocks stored for both matrices.
    Output: 2048 fp16 cols/batch = 4.2 MB/core (vs 5.24).
  - PSUM evictions split DVE / ACT (v1 had all on DVE at 1x PSUM mode).
  - -a (for the a^T c matmul) built on DVE in fp16 SBUF (4x mode).
Per (batch, mi-rowblock): banks PR = sum_kt a^T a + c^T c,
PI = sum_kt c^T a + (-a)^T c; 4 fp16 matmuls per (mi, kt), exact f32 PSUM
accumulation. PE ~34 us/core is the predicted bottleneck (DMA ~24 us,
DVE ~20 us, ACT ~18 us).
"""
import numpy as np
from contextlib import ExitStack

import concourse.bacc as bacc
import concourse.tile as tile
from concourse import mybir
from concourse.bass_utils import run_bass_kernel_spmd

F32 = mybir.dt.float32
FP16 = mybir.dt.float16
FP8 = mybir.dt.float8e4

N_CORES = 8
B_FULL = 64
BPC = B_FULL // N_CORES  # batches per core
T, D = 256, 512
KT = T // 128             # K tiles per batch
MT = D // 128              # output row blocks

# per-batch packed output layout (fp16 cols):
#   [Ro offdiag (768) | Io offdiag (768) | M diag blocks (512)] = 2048
RO_W = [D - 128 * (mi + 1) for mi in range(MT)]      # 384, 256, 128, 0
RO_OFF = [sum(RO_W[:j]) for j in range(MT)]          # 0, 384, 640, 768
SEG_IO = sum(RO_W)                                   # 768
SEG_M = 2 * SEG_IO                                   # 1536
PB = SEG_M + MT * 128                                # 2048 cols per batch
IN_PB = KT * 2 * D                                   # 2048 input cols per batch
CHB = 2                                              # batches per DMA chunk


ALL_PARTS = frozenset({"load", "neg", "mm", "evict", "store"})


def build_nc(reps: int = 1, unroll: int = 1,
             ld_bufs=3, na_bufs=4, st_bufs=3, tmp_bufs=8, ps_bufs=8,
             parts=ALL_PARTS, chunks=(1,) * BPC):
    """Build + compile the per-core program. reps>1 wraps the body in a
    hardware loop (timing only; output idempotent). unroll>1 python-unrolls
    (for the timeline simulator, which can't run For_i). parts: ablation
    subsets for bench.py (timing experiments only)."""
    nc = bacc.Bacc("TRN2", target_bir_lowering=False, debug=False)
    inp = nc.dram_tensor("inp", [128, BPC * IN_PB], FP16,
                         kind="ExternalInput").ap()
    outp = nc.dram_tensor("outp", [128, BPC * PB], FP16,
                          kind="ExternalOutput").ap()
    warm_out = nc.dram_tensor("warm", [128, 8], FP16,
                              kind="ExternalOutput").ap()

    with tile.TileContext(nc) as tc, ExitStack() as ctx:
        ld = ctx.enter_context(tc.tile_pool(name="ld", bufs=ld_bufs))
        nap = ctx.enter_context(tc.tile_pool(name="nap", bufs=na_bufs))
        stp = ctx.enter_context(tc.tile_pool(name="stp", bufs=st_bufs))
        tmp = ctx.enter_context(tc.tile_pool(name="tmp", bufs=tmp_bufs))
        psp = ctx.enter_context(tc.tile_pool(name="psp", bufs=ps_bufs, space="PSUM"))

        def body(_iv=None):
            assert sum(chunks) == BPC
            b0s = [sum(chunks[:j]) for j in range(len(chunks))]
            for ch, (b0, nb) in enumerate(zip(b0s, chunks)):
                cht = ld.tile([128, CHB * IN_PB], FP16, tag="ch")
                if "load" in parts:
                    eng = nc.sync if ch % 2 == 0 else nc.scalar
                    eng2 = nc.scalar if ch % 2 == 0 else nc.sync
                    if ch == 0:
                        # split the first load (a-kt0 | c-kt0 | rest) so
                        # batch 0's first matmuls start ~1.7 us sooner
                        i0 = b0 * IN_PB
                        eng.dma_start(cht[:, :D], inp[:, i0:i0 + D])
                        eng2.dma_start(cht[:, D:2 * D], inp[:, i0 + D:i0 + 2 * D])
                        eng.dma_start(cht[:, 2 * D:nb * IN_PB],
                                      inp[:, i0 + 2 * D:i0 + nb * IN_PB])
                    else:
                        eng.dma_start(cht[:, :nb * IN_PB],
                                      inp[:, b0 * IN_PB:(b0 + nb) * IN_PB])

                st = stp.tile([128, CHB * PB], FP16, tag="st")
                for bi in range(nb):
                    boff = bi * IN_PB
                    mms = {kt: [] for kt in range(KT)}
                    evs = []
                    kt_outer = ch == 0 and bi == 0
                    # negate a (fp16 SBUF, 4x DVE) for the -a^T c matmul;
                    # stationary slices for (mi, kt) union to all of a.
                    na_full = nap.tile([128, KT * D], FP16, tag="naf")
                    if "neg" in parts:
                        for kt in range(KT):
                            nc.vector.tensor_scalar_mul(
                                na_full[:, kt * D:(kt + 1) * D],
                                cht[:, boff + kt * 2 * D: boff + kt * 2 * D + D],
                                -1.0)

                    seg = bi * PB
                    for mi in range(MT):
                        w = D - mi * 128
                        col0 = mi * 128
                        PR = psp.tile([128, w], F32, tag="ps",
                                      padded_shape=[128, D], name="pr")
                        PI = psp.tile([128, w], F32, tag="ps",
                                      padded_shape=[128, D], name="pi")
                        if "mm8" in parts:
                            # timing-shape probe: 12 fp8 DoubleRow matmuls
                            # (K=256 each) as the hi/lo-compensated scheme
                            # would issue. Operand values are garbage
                            # (bitcast of the fp16 chunk); timing-valid.
                            DR = mybir.MatmulPerfMode.DoubleRow
                            c8 = cht[:, boff:boff + IN_PB].bitcast(FP8)
                            n8 = na_full[:].bitcast(FP8)

                            def sl8(base, c0, n):
                                return base.rearrange(
                                    "p (k d) -> p k d", k=2)[:, :, c0:c0 + n]

                            sbases = [0, 512, 1024, 1536]  # ah, al, ch, cl
                            stats = [sl8(c8, sbases[j % 4] + col0, 128)
                                     if j % 3 else sl8(n8, (j % 2) * 512 + col0, 128)
                                     for j in range(12)]
                            movs = [sl8(c8, sbases[(j + 1) % 4] + col0, w)
                                    for j in range(12)]
                            banks = [PR, PI] * 6
                            for j in range(12):
                                nc.tensor.matmul(banks[j][:], stats[j], movs[j],
                                                 start=j < 2, stop=j >= 10,
                                                 perf_mode=DR)
                        elif "mm" in parts:
                            for kt in range(KT):
                                ak = boff + kt * 2 * D          # a cols base
                                ck = ak + D                      # c cols base
                                st_ = kt == 0
                                sp = kt == KT - 1
                                a_m = cht[:, ak + col0: ak + col0 + 128]
                                a_n = cht[:, ak + col0: ak + col0 + w]
                                c_m = cht[:, ck + col0: ck + col0 + 128]
                                c_n = cht[:, ck + col0: ck + col0 + w]
                                na_m = na_full[:, kt * D + col0: kt * D + col0 + 128]
                                mms[kt].append(
                                    (PR, a_m, a_n, st_, False))
                                mms[kt].append(
                                    (PI, c_m, a_n, st_, False))
                                mms[kt].append(
                                    (PI, na_m, c_n, False, sp))
                                mms[kt].append(
                                    (PR, c_m, c_n, False, sp))
                        # evictions: off-diag Ro on DVE, Io on ACT;
                        # diag: M = PR_d + PI_d (ACT copies PI_d to SBUF f32,
                        # DVE adds PSUM+SBUF -> fp16)
                        if "evict" in parts:
                            def ev(PR=PR, PI=PI, mi=mi, w=w, seg=seg):
                                if w > 128:
                                    nc.vector.tensor_copy(
                                        st[:, seg + RO_OFF[mi]: seg + RO_OFF[mi] + w - 128],
                                        PR[:, 128:w])
                                    nc.scalar.mul(
                                        st[:, seg + SEG_IO + RO_OFF[mi]:
                                           seg + SEG_IO + RO_OFF[mi] + w - 128],
                                        PI[:, 128:w], 1.0)
                                td = tmp.tile([128, 128], F32, tag="td", name=f"td{mi}")
                                nc.scalar.mul(td[:], PI[:, 0:128], 1.0)
                                nc.vector.tensor_add(
                                    st[:, seg + SEG_M + mi * 128: seg + SEG_M + (mi + 1) * 128],
                                    PR[:, 0:128], td[:])
                            evs.append(ev)

                    def issue(bank, l, r, st_, sp):
                        nc.tensor.matmul(bank[:], l, r, start=st_, stop=sp)

                    if kt_outer:
                        # a-only matmuls first (they need just the first
                        # 512-col load), then the rest of kt0, then kt1
                        first = [m for j, m in enumerate(mms[0]) if j % 4 == 0]
                        rest = [m for j, m in enumerate(mms[0]) if j % 4]
                        for m in first + rest + mms[1]:
                            issue(*m)
                        for ev in evs:
                            ev()
                    else:
                        for mi in range(MT):
                            for kt in range(KT):
                                for m in mms[kt][mi * 4:(mi + 1) * 4]:
                                    issue(*m)
                            if evs:
                                evs[mi]()
                if "store" in parts:
                    if ch == len(chunks) - 1:
                        # split the final store: Ro+Io (ready after mi2's
                        # eviction) early, the small M segment at the end
                        cut = (nb - 1) * PB + SEG_M
                        nc.gpsimd.dma_start(
                            outp[:, b0 * PB:b0 * PB + cut], st[:, :cut])
                        nc.gpsimd.dma_start(
                            outp[:, b0 * PB + cut:(b0 + nb) * PB],
                            st[:, cut:nb * PB])
                    else:
                        nc.gpsimd.dma_start(
                            outp[:, b0 * PB:(b0 + nb) * PB], st[:, :nb * PB])

        def warmup(n_mm=12):
            # PE p-state warmup: dummy matmuls (uninitialized SBUF operands,
            # discarded results) run while the first input DMA is in flight,
            # so real matmuls start at the full 2.4 GHz clock. Outside the
            # timing loop: steady-state cost is zero.
            wsrc = nap.tile([128, 2 * D], FP16, tag="wsrc")
            nc.vector.memset(wsrc[:], 0.0)
            wps = psp.tile([128, D], F32, tag="ps", padded_shape=[128, D],
                           name="wps")
            for j in range(n_mm):
                nc.tensor.matmul(wps[:], wsrc[:, 0:128], wsrc[:, D:2 * D],
                                 start=j == 0, stop=j == n_mm - 1)
            wo = nap.tile([128, 8], FP16, tag="wo")
            nc.vector.tensor_copy(wo[:], wps[:, 0:8])
            nc.gpsimd.dma_start(warm_out, wo[:])

        if "warm" not in parts:
            pass
        elif unroll <= 1:
            warmup()
        if unroll > 1:
            for _ in range(unroll):
                body()
        elif reps == 1:
            body()
        else:
            with tc.For_i(0, reps, 1) as iv:
                body(iv)

    nc.compile()
    return nc


_NC_CACHE = {}


def _get_nc(reps: int = 1):
    if reps not in _NC_CACHE:
        _NC_CACHE[reps] = build_nc(reps=reps)
    return _NC_CACHE[reps]


def make_in_maps(real, imag, weight):
    """Host prescale + pack: per core IN[128, BPC*IN_PB] fp16 where
    IN[p, ((b*KT + kt)*2 + role)*D + d] = fp16(sqrt(w[b,kt*128+p]) *
    {real,imag}[b, kt*128+p, d])."""
    sw = np.sqrt(weight.astype(np.float32))           # [B, T, 1]
    a = (sw * real).astype(np.float16).reshape(B_FULL, KT, 128, D)
    c = (sw * imag).astype(np.float16).reshape(B_FULL, KT, 128, D)
    ac = np.stack([a, c], axis=3)                     # [B, KT, 128, 2, D]
    maps = []
    for k in range(N_CORES):
        sub = ac[k * BPC:(k + 1) * BPC]               # [BPC, KT, 128, 2, D]
        x = np.ascontiguousarray(sub.transpose(2, 0, 1, 3, 4)
                                 ).reshape(128, BPC * IN_PB)
        maps.append({"inp": x})
    return maps


def _unpack(res_list):
    """Per-core outp [128, BPC*PB] fp16 -> full f32 (out_r, out_i)."""
    p = np.stack(res_list, axis=0).astype(np.float32)   # [NC, 128, BPC*PB]
    p = p.reshape(N_CORES, 128, BPC, PB).transpose(0, 2, 1, 3)
    p = p.reshape(B_FULL, 128, PB)                      # [B, 128, PB]
    out_r = np.empty((B_FULL, D, D), np.float32)
    out_i = np.empty((B_FULL, D, D), np.float32)
    for mi in range(MT):
        rs = slice(mi * 128, (mi + 1) * 128)
        # diag block: M = D_r + D_i
        M = p[:, :, SEG_M + mi * 128: SEG_M + (mi + 1) * 128]
        Mt = M.transpose(0, 2, 1)
        out_r[:, rs, rs] = (M + Mt) * 0.5
        out_i[:, rs, rs] = (M - Mt) * 0.5
        w = RO_W[mi]
        if w:
            cs = slice((mi + 1) * 128, D)
            out_r[:, rs, cs] = p[:, :, RO_OFF[mi]: RO_OFF[mi] + w]
            out_i[:, rs, cs] = p[:, :, SEG_IO + RO_OFF[mi]: SEG_IO + RO_OFF[mi] + w]
            # mirror lower blocks
            out_r[:, cs, rs] = out_r[:, rs, cs].transpose(0, 2, 1)
            out_i[:, cs, rs] = -out_i[:, rs, cs].transpose(0, 2, 1)
    return out_r, out_i


def kernel(real, imag, weight):
    real = np.asarray(real, dtype=np.float32)
    imag = np.asarray(imag, dtype=np.float32)
    weight = np.asarray(weight, dtype=np.float32)
    assert real.shape == (B_FULL, T, D) and weight.shape == (B_FULL, T, 1)

    nc = _get_nc()
    in_maps = make_in_maps(real, imag, weight)
    res = run_bass_kernel_spmd(nc, in_maps, list(range(N_CORES)))
    return _unpack([res.results[i]["outp"] for i in range(N_CORES)])


# revision 23
# speedup vs baseline: 1.6081x; 1.6081x over previous
"""Trainium2 Bass kernel for nn_ComplexMixture: weighted complex Gram matrices.

Reference (per batch b, inputs real/imag [B,T,D] f32, weight [B,T,1] f32):
    out_r[b] = sum_t w[b,t] * (r_t r_t^T + i_t i_t^T)   (symmetric)
    out_i[b] = sum_t w[b,t] * (i_t r_t^T - r_t i_t^T)   (antisymmetric)
with B=64, T=256, D=512; outputs (out_r, out_i), each [B, D, D] f32.

Pure data-parallel over 8 NeuronCores (8 batches per core).

v2 design (vs v1 at ~143 us/iter):
  - Host pre-scales: a = fp16(sqrt(w)*r), c = fp16(sqrt(w)*i). Removes the
    on-device weight DMA (2048 4-byte descriptors!), the ACT scaling stage,
    and halves input DMA bytes (f32 -> fp16).
  - Host pre-transposes into IN[128, BPC*KT*2*D] (partition = time-within-
    K-tile) so input DMA is 4x 1MB chunks with 128x8KB fully contiguous
    descriptors (line rate ~341 GB/s), alternated across the two HWDGE
    rings (nc.sync / nc.scalar).
  - Diagonal-block combine: only M = D_r + D_i is stored for the 4 diagonal
    blocks (host recovers D_r=(M+M^T)/2, D_i=(M-M^T)/2 since D_r symmetric,
    D_i antisymmetric). Off-diag upper blocks stored for both matrices.
    Output: 2048 fp16 cols/batch = 4.2 MB/core (vs 5.24).
  - PSUM evictions split DVE / ACT (v1 had all on DVE at 1x PSUM mode).
  - -a (for the a^T c matmul) built on DVE in fp16 SBUF (4x mode).
Per (batch, mi-rowblock): banks PR = sum_kt a^T a + c^T c,
PI = sum_kt c^T a + (-a)^T c; 4 fp16 matmuls per (mi, kt), exact f32 PSUM
accumulation. PE ~34 us/core is the predicted bottleneck (DMA ~24 us,
DVE ~20 us, ACT ~18 us).
"""
import numpy as np
from contextlib import ExitStack

import concourse.bacc as bacc
import concourse.tile as tile
from concourse import mybir
from concourse.bass_utils import run_bass_kernel_spmd

F32 = mybir.dt.float32
FP16 = mybir.dt.float16
FP8 = mybir.dt.float8e4

N_CORES = 8
B_FULL = 64
BPC = B_FULL // N_CORES  # batches per core
T, D = 256, 512
KT = T // 128             # K tiles per batch
MT = D // 128              # output row blocks

# per-batch packed output layout (fp16 cols):
#   [Ro offdiag (768) | Io offdiag (768) | M diag blocks (512)] = 2048
RO_W = [D - 128 * (mi + 1) for mi in range(MT)]      # 384, 256, 128, 0
RO_OFF = [sum(RO_W[:j]) for j in range(MT)]          # 0, 384, 640, 768
SEG_IO = sum(RO_W)                                   # 768
SEG_M = 2 * SEG_IO                                   # 1536
PB = SEG_M + MT * 128                                # 2048 cols per batch
IN_PB = KT * 2 * D                                   # 2048 input cols per batch
CHB = 2                                              # batches per DMA chunk


ALL_PARTS = frozenset({"load", "neg", "mm", "evict", "store"})


def build_nc(reps: int = 1, unroll: int = 1,
             ld_bufs=3, na_bufs=4, st_bufs=3, tmp_bufs=8, ps_bufs=8,
             parts=ALL_PARTS, chunks=(1,) * BPC,
             head_split=True, tail_split=True):
    """Build + compile the per-core program. reps>1 wraps the body in a
    hardware loop (timing only; output idempotent). unroll>1 python-unrolls
    (for the timeline simulator, which can't run For_i). parts: ablation
    subsets for bench.py (timing experiments only)."""
    nc = bacc.Bacc("TRN2", target_bir_lowering=False, debug=False)
    inp = nc.dram_tensor("inp", [128, BPC * IN_PB], FP16,
                         kind="ExternalInput").ap()
    outp = nc.dram_tensor("outp", [128, BPC * PB], FP16,
                          kind="ExternalOutput").ap()
    warm_out = nc.dram_tensor("warm", [128, 8], FP16,
                              kind="ExternalOutput").ap()

    with tile.TileContext(nc) as tc, ExitStack() as ctx:
        ld = ctx.enter_context(tc.tile_pool(name="ld", bufs=ld_bufs))
        nap = ctx.enter_context(tc.tile_pool(name="nap", bufs=na_bufs))
        stp = ctx.enter_context(tc.tile_pool(name="stp", bufs=st_bufs))
        tmp = ctx.enter_context(tc.tile_pool(name="tmp", bufs=tmp_bufs))
        psp = ctx.enter_context(tc.tile_pool(name="psp", bufs=ps_bufs, space="PSUM"))

        def body(_iv=None):
            assert sum(chunks) == BPC
            nbmax = max(chunks)
            b0s = [sum(chunks[:j]) for j in range(len(chunks))]

            def load_chunk(ch, b0, nb):
                # load + build this chunk's -a tiles; called one chunk
                # ahead of the matmuls so the PE never waits on the DVE
                # queue behind older evictions (priority inversion)
                cht = ld.tile([128, nbmax * IN_PB], FP16, tag="ch",
                              name=f"cht{ch}")
                if "load" in parts:
                    eng = nc.sync if ch % 2 == 0 else nc.scalar
                    eng2 = nc.scalar if ch % 2 == 0 else nc.sync
                    if ch == 0 and head_split:
                        # split the first load (a-kt0 | c-kt0 | rest) so
                        # batch 0's first matmuls start ~1.7 us sooner
                        i0 = b0 * IN_PB
                        eng.dma_start(cht[:, :D], inp[:, i0:i0 + D])
                        eng2.dma_start(cht[:, D:2 * D], inp[:, i0 + D:i0 + 2 * D])
                        eng.dma_start(cht[:, 2 * D:nb * IN_PB],
                                      inp[:, i0 + 2 * D:i0 + nb * IN_PB])
                    else:
                        eng.dma_start(cht[:, :nb * IN_PB],
                                      inp[:, b0 * IN_PB:(b0 + nb) * IN_PB])
                nal = []
                for bi in range(nb):
                    na_full = nap.tile([128, KT * D], FP16, tag="naf",
                                       name=f"na{ch}_{bi}")
                    if "neg" in parts:
                        boff = bi * IN_PB
                        for kt in range(KT):
                            nc.vector.tensor_scalar_mul(
                                na_full[:, kt * D:(kt + 1) * D],
                                cht[:, boff + kt * 2 * D: boff + kt * 2 * D + D],
                                -1.0)
                    nal.append(na_full)
                return cht, nal

            chts = {0: load_chunk(0, b0s[0], chunks[0])}
            for ch, (b0, nb) in enumerate(zip(b0s, chunks)):
                cht, nal = chts.pop(ch)
                if ch + 1 < len(chunks):
                    chts[ch + 1] = load_chunk(ch + 1, b0s[ch + 1], chunks[ch + 1])

                st = stp.tile([128, nbmax * PB], FP16, tag="st")
                for bi in range(nb):
                    boff = bi * IN_PB
                    mms = {kt: [] for kt in range(KT)}
                    evs = []
                    kt_outer = ch == 0 and bi == 0 and head_split
                    na_full = nal[bi]

                    seg = bi * PB
                    for mi in range(MT):
                        w = D - mi * 128
                        col0 = mi * 128
                        PR = psp.tile([128, w], F32, tag="ps",
                                      padded_shape=[128, D], name="pr")
                        PI = psp.tile([128, w], F32, tag="ps",
                                      padded_shape=[128, D], name="pi")
                        if "mm8" in parts:
                            # timing-shape probe: 12 fp8 DoubleRow matmuls
                            # (K=256 each) as the hi/lo-compensated scheme
                            # would issue. Operand values are garbage
                            # (bitcast of the fp16 chunk); timing-valid.
                            DR = mybir.MatmulPerfMode.DoubleRow
                            c8 = cht[:, boff:boff + IN_PB].bitcast(FP8)
                            n8 = na_full[:].bitcast(FP8)

                            def sl8(base, c0, n):
                                return base.rearrange(
                                    "p (k d) -> p k d", k=2)[:, :, c0:c0 + n]

                            sbases = [0, 512, 1024, 1536]  # ah, al, ch, cl
                            stats = [sl8(c8, sbases[j % 4] + col0, 128)
                                     if j % 3 else sl8(n8, (j % 2) * 512 + col0, 128)
                                     for j in range(12)]
                            movs = [sl8(c8, sbases[(j + 1) % 4] + col0, w)
                                    for j in range(12)]
                            banks = [PR, PI] * 6
                            for j in range(12):
                                nc.tensor.matmul(banks[j][:], stats[j], movs[j],
                                                 start=j < 2, stop=j >= 10,
                                                 perf_mode=DR)
                        elif "mm" in parts:
                            for kt in range(KT):
                                ak = boff + kt * 2 * D          # a cols base
                                ck = ak + D                      # c cols base
                                st_ = kt == 0
                                sp = kt == KT - 1
                                a_m = cht[:, ak + col0: ak + col0 + 128]
                                a_n = cht[:, ak + col0: ak + col0 + w]
                                c_m = cht[:, ck + col0: ck + col0 + 128]
                                c_n = cht[:, ck + col0: ck + col0 + w]
                                na_m = na_full[:, kt * D + col0: kt * D + col0 + 128]
                                mms[kt].append(
                                    (PR, a_m, a_n, st_, False))
                                mms[kt].append(
                                    (PI, c_m, a_n, st_, False))
                                mms[kt].append(
                                    (PI, na_m, c_n, False, sp))
                                mms[kt].append(
                                    (PR, c_m, c_n, False, sp))
                        # evictions: off-diag Ro on DVE, Io on ACT;
                        # diag: M = PR_d + PI_d (ACT copies PI_d to SBUF f32,
                        # DVE adds PSUM+SBUF -> fp16)
                        if "evict" in parts:
                            def ev(PR=PR, PI=PI, mi=mi, w=w, seg=seg):
                                if w > 128:
                                    nc.vector.tensor_copy(
                                        st[:, seg + RO_OFF[mi]: seg + RO_OFF[mi] + w - 128],
                                        PR[:, 128:w])
                                    nc.scalar.mul(
                                        st[:, seg + SEG_IO + RO_OFF[mi]:
                                           seg + SEG_IO + RO_OFF[mi] + w - 128],
                                        PI[:, 128:w], 1.0)
                                td = tmp.tile([128, 128], F32, tag="td", name=f"td{mi}")
                                nc.scalar.mul(td[:], PI[:, 0:128], 1.0)
                                nc.vector.tensor_add(
                                    st[:, seg + SEG_M + mi * 128: seg + SEG_M + (mi + 1) * 128],
                                    PR[:, 0:128], td[:])
                            evs.append(ev)

                    def issue(bank, l, r, st_, sp):
                        nc.tensor.matmul(bank[:], l, r, start=st_, stop=sp)

                    if kt_outer:
                        # a-only matmuls first (they need just the first
                        # 512-col load), then the rest of kt0, then kt1
                        first = [m for j, m in enumerate(mms[0]) if j % 4 == 0]
                        rest = [m for j, m in enumerate(mms[0]) if j % 4]
                        for m in first + rest + mms[1]:
                            issue(*m)
                        for ev in evs:
                            ev()
                    else:
                        for mi in range(MT):
                            for kt in range(KT):
                                for m in mms[kt][mi * 4:(mi + 1) * 4]:
                                    issue(*m)
                            if evs:
                                evs[mi]()
                if "store" in parts:
                    if ch == len(chunks) - 1 and tail_split:
                        # split the final store: Ro+Io (ready after mi2's
                        # eviction) early, the small M segment at the end
                        cut = (nb - 1) * PB + SEG_M
                        nc.gpsimd.dma_start(
                            outp[:, b0 * PB:b0 * PB + cut], st[:, :cut])
                        nc.gpsimd.dma_start(
                            outp[:, b0 * PB + cut:(b0 + nb) * PB],
                            st[:, cut:nb * PB])
                    else:
                        nc.gpsimd.dma_start(
                            outp[:, b0 * PB:(b0 + nb) * PB], st[:, :nb * PB])

        def warmup(n_mm=12):
            # PE p-state warmup: dummy matmuls (uninitialized SBUF operands,
            # discarded results) run while the first input DMA is in flight,
            # so real matmuls start at the full 2.4 GHz clock. Outside the
            # timing loop: steady-state cost is zero.
            wsrc = nap.tile([128, 2 * D], FP16, tag="wsrc")
            nc.vector.memset(wsrc[:], 0.0)
            wps = psp.tile([128, D], F32, tag="ps", padded_shape=[128, D],
                           name="wps")
            for j in range(n_mm):
                nc.tensor.matmul(wps[:], wsrc[:, 0:128], wsrc[:, D:2 * D],
                                 start=j == 0, stop=j == n_mm - 1)
            wo = nap.tile([128, 8], FP16, tag="wo")
            nc.vector.tensor_copy(wo[:], wps[:, 0:8])
            nc.gpsimd.dma_start(warm_out, wo[:])

        if "warm" not in parts:
            pass
        elif unroll <= 1:
            warmup()
        if unroll > 1:
            for _ in range(unroll):
                body()
        elif reps == 1:
            body()
        else:
            with tc.For_i(0, reps, 1) as iv:
                body(iv)

    nc.compile()
    return nc


_NC_CACHE = {}


def _get_nc(reps: int = 1):
    if reps not in _NC_CACHE:
        _NC_CACHE[reps] = build_nc(reps=reps)
    return _NC_CACHE[reps]


def make_in_maps(real, imag, weight):
    """Host prescale + pack: per core IN[128, BPC*IN_PB] fp16 where
    IN[p, ((b*KT + kt)*2 + role)*D + d] = fp16(sqrt(w[b,kt*128+p]) *
    {real,imag}[b, kt*128+p, d])."""
    sw = np.sqrt(weight.astype(np.float32))           # [B, T, 1]
    a = (sw * real).astype(np.float16).reshape(B_FULL, KT, 128, D)
    c = (sw * imag).astype(np.float16).reshape(B_FULL, KT, 128, D)
    ac = np.stack([a, c], axis=3)                     # [B, KT, 128, 2, D]
    maps = []
    for k in range(N_CORES):
        sub = ac[k * BPC:(k + 1) * BPC]               # [BPC, KT, 128, 2, D]
        x = np.ascontiguousarray(sub.transpose(2, 0, 1, 3, 4)
                                 ).reshape(128, BPC * IN_PB)
        maps.append({"inp": x})
    return maps


def _unpack(res_list):
    """Per-core outp [128, BPC*PB] fp16 -> full f32 (out_r, out_i)."""
    p = np.stack(res_list, axis=0).astype(np.float32)   # [NC, 128, BPC*PB]
    p = p.reshape(N_CORES, 128, BPC, PB).transpose(0, 2, 1, 3)
    p = p.reshape(B_FULL, 128, PB)                      # [B, 128, PB]
    out_r = np.empty((B_FULL, D, D), np.float32)
    out_i = np.empty((B_FULL, D, D), np.float32)
    for mi in range(MT):
        rs = slice(mi * 128, (mi + 1) * 128)
        # diag block: M = D_r + D_i
        M = p[:, :, SEG_M + mi * 128: SEG_M + (mi + 1) * 128]
        Mt = M.transpose(0, 2, 1)
        out_r[:, rs, rs] = (M + Mt) * 0.5
        out_i[:, rs, rs] = (M - Mt) * 0.5
        w = RO_W[mi]
        if w:
            cs = slice((mi + 1) * 128, D)
            out_r[:, rs, cs] = p[:, :, RO_OFF[mi]: RO_OFF[mi] + w]
            out_i[:, rs, cs] = p[:, :, SEG_IO + RO_OFF[mi]: SEG_IO + RO_OFF[mi] + w]
            # mirror lower blocks
            out_r[:, cs, rs] = out_r[:, rs, cs].transpose(0, 2, 1)
            out_i[:, cs, rs] = -out_i[:, rs, cs].transpose(0, 2, 1)
    return out_r, out_i


def kernel(real, imag, weight):
    real = np.asarray(real, dtype=np.float32)
    imag = np.asarray(imag, dtype=np.float32)
    weight = np.asarray(weight, dtype=np.float32)
    assert real.shape == (B_FULL, T, D) and weight.shape == (B_FULL, T, 1)

    nc = _get_nc()
    in_maps = make_in_maps(real, imag, weight)
    res = run_bass_kernel_spmd(nc, in_maps, list(range(N_CORES)))
    return _unpack([res.results[i]["outp"] for i in range(N_CORES)])


# revision 24
# speedup vs baseline: 1.6171x; 1.0056x over previous
"""Trainium2 Bass kernel for nn_ComplexMixture: weighted complex Gram matrices.

Reference (per batch b, inputs real/imag [B,T,D] f32, weight [B,T,1] f32):
    out_r[b] = sum_t w[b,t] * (r_t r_t^T + i_t i_t^T)   (symmetric)
    out_i[b] = sum_t w[b,t] * (i_t r_t^T - r_t i_t^T)   (antisymmetric)
with B=64, T=256, D=512; outputs (out_r, out_i), each [B, D, D] f32.

Pure data-parallel over 8 NeuronCores (8 batches per core). Final design:
  - Host pre-scales and packs: a = fp16(sqrt(w)*r), c = fp16(sqrt(w)*i),
    transposed to IN[128, (b,kt,role,d)] so every input DMA is fully
    contiguous per partition. Removes all on-device weight handling (v1
    spent ~1/3 of its time on a 2048x4B-descriptor weight DMA + ACT
    scaling) and halves input bytes. fp16 operands, exact f32 PSUM
    accumulation: rel err ~3.6e-4 vs the 2e-2 gate.
  - Only the upper block-trapezoid is computed (out_r symmetric, out_i
    antisymmetric; host mirrors the lower blocks): per (batch, mi row-
    block): PR = sum_kt a^T a + c^T c, PI = sum_kt c^T a + (-a)^T c,
    4 fp16 matmuls per (mi, kt) = 32 per batch. PE streaming is the
    roofline term: 81920 PE columns/core = 34.1 us at 2.4 GHz.
  - Diagonal-block combine: only M = D_r + D_i is stored for the 4
    diagonal blocks (host recovers D_r=(M+M^T)/2, D_i=(M-M^T)/2).
    Output 2048 fp16 cols/batch = 4.2 MB/core.
  - -a (for the -a^T c matmul) built on DVE (fp16 4x mode), loaded+built
    one chunk AHEAD of the matmuls (software pipeline) so the PE does not
    wait on the DVE queue behind the previous batch's evictions.
  - PSUM evictions split DVE / ACT; 8 single-bank PSUM tiles rotate so
    ~4 row-blocks are in flight across batch boundaries.
  - Loads: one 512KB DMA per batch, alternating the two HWDGE rings
    (sync/scalar). Measured: finer chunks beat coarser (50.7 vs 56/64 us
    for 2/4-batch chunks) - DMA fixed costs hide better and the na
    pipeline stays tight. First load split (a-kt0 | c-kt0 | rest) so
    batch 0's matmuls start ~2 us sooner (single-shot head). Stores per
    batch via SWDGE (gpsimd); last store split so the tail transfer
    after the final eviction is short.
Host: unpack fp16 -> f32, mirror lower blocks (r: +T, i: -T).

Measured (device-resident PJRT differencing, reps 1 vs 2049, median of
alternating rounds): ~50.8 us/iter vs ~143-172 us for the v1 baseline.
Engine budget at that point: PE ~34 us streaming + ~10 us of loop/sync
overhead (p-state ramps after small stalls), loads ~19 us and stores
~15 us largely hidden. Rejected after measurement: fp8e4 DoubleRow
matmuls (2x stream rate but 256-row weight loads do not overlap: 2.65x
SLOWER end-to-end); consolidated 6-op evictions + full PR/PI store
(v4: +1 MB output ate the op savings); PE warmup chain (sim: worse).
"""
import numpy as np
from contextlib import ExitStack

import concourse.bacc as bacc
import concourse.tile as tile
from concourse import mybir
from concourse.bass_utils import run_bass_kernel_spmd

F32 = mybir.dt.float32
FP16 = mybir.dt.float16
FP8 = mybir.dt.float8e4

N_CORES = 8
B_FULL = 64
BPC = B_FULL // N_CORES  # batches per core
T, D = 256, 512
KT = T // 128             # K tiles per batch
MT = D // 128              # output row blocks

# per-batch packed output layout (fp16 cols):
#   [Ro offdiag (768) | Io offdiag (768) | M diag blocks (512)] = 2048
RO_W = [D - 128 * (mi + 1) for mi in range(MT)]      # 384, 256, 128, 0
RO_OFF = [sum(RO_W[:j]) for j in range(MT)]          # 0, 384, 640, 768
SEG_IO = sum(RO_W)                                   # 768
SEG_M = 2 * SEG_IO                                   # 1536
PB = SEG_M + MT * 128                                # 2048 cols per batch
IN_PB = KT * 2 * D                                   # 2048 input cols per batch
CHB = 2                                              # batches per DMA chunk


ALL_PARTS = frozenset({"load", "neg", "mm", "evict", "store"})


def build_nc(reps: int = 1, unroll: int = 1,
             ld_bufs=3, na_bufs=4, st_bufs=3, tmp_bufs=8, ps_bufs=8,
             parts=ALL_PARTS, chunks=(1,) * BPC,
             head_split=True, tail_split=True):
    """Build + compile the per-core program. reps>1 wraps the body in a
    hardware loop (timing only; output idempotent). unroll>1 python-unrolls
    (for the timeline simulator, which can't run For_i). parts: ablation
    subsets for bench.py (timing experiments only)."""
    nc = bacc.Bacc("TRN2", target_bir_lowering=False, debug=False)
    inp = nc.dram_tensor("inp", [128, BPC * IN_PB], FP16,
                         kind="ExternalInput").ap()
    outp = nc.dram_tensor("outp", [128, BPC * PB], FP16,
                          kind="ExternalOutput").ap()
    warm_out = nc.dram_tensor("warm", [128, 8], FP16,
                              kind="ExternalOutput").ap()

    with tile.TileContext(nc) as tc, ExitStack() as ctx:
        ld = ctx.enter_context(tc.tile_pool(name="ld", bufs=ld_bufs))
        nap = ctx.enter_context(tc.tile_pool(name="nap", bufs=na_bufs))
        stp = ctx.enter_context(tc.tile_pool(name="stp", bufs=st_bufs))
        tmp = ctx.enter_context(tc.tile_pool(name="tmp", bufs=tmp_bufs))
        psp = ctx.enter_context(tc.tile_pool(name="psp", bufs=ps_bufs, space="PSUM"))

        def body(_iv=None):
            assert sum(chunks) == BPC
            nbmax = max(chunks)
            b0s = [sum(chunks[:j]) for j in range(len(chunks))]

            def load_chunk(ch, b0, nb):
                # load + build this chunk's -a tiles; called one chunk
                # ahead of the matmuls so the PE never waits on the DVE
                # queue behind older evictions (priority inversion)
                cht = ld.tile([128, nbmax * IN_PB], FP16, tag="ch",
                              name=f"cht{ch}")
                if "load" in parts:
                    eng = nc.sync if ch % 2 == 0 else nc.scalar
                    eng2 = nc.scalar if ch % 2 == 0 else nc.sync
                    if ch == 0 and head_split:
                        # split the first load (a-kt0 | c-kt0 | rest) so
                        # batch 0's first matmuls start ~1.7 us sooner
                        i0 = b0 * IN_PB
                        eng.dma_start(cht[:, :D], inp[:, i0:i0 + D])
                        eng2.dma_start(cht[:, D:2 * D], inp[:, i0 + D:i0 + 2 * D])
                        eng.dma_start(cht[:, 2 * D:nb * IN_PB],
                                      inp[:, i0 + 2 * D:i0 + nb * IN_PB])
                    else:
                        eng.dma_start(cht[:, :nb * IN_PB],
                                      inp[:, b0 * IN_PB:(b0 + nb) * IN_PB])
                nal = []
                for bi in range(nb):
                    na_full = nap.tile([128, KT * D], FP16, tag="naf",
                                       name=f"na{ch}_{bi}")
                    if "neg" in parts:
                        boff = bi * IN_PB
                        for kt in range(KT):
                            nc.vector.tensor_scalar_mul(
                                na_full[:, kt * D:(kt + 1) * D],
                                cht[:, boff + kt * 2 * D: boff + kt * 2 * D + D],
                                -1.0)
                    nal.append(na_full)
                return cht, nal

            chts = {0: load_chunk(0, b0s[0], chunks[0])}
            for ch, (b0, nb) in enumerate(zip(b0s, chunks)):
                cht, nal = chts.pop(ch)
                if ch + 1 < len(chunks):
                    chts[ch + 1] = load_chunk(ch + 1, b0s[ch + 1], chunks[ch + 1])

                st = stp.tile([128, nbmax * PB], FP16, tag="st")
                for bi in range(nb):
                    boff = bi * IN_PB
                    mms = {kt: [] for kt in range(KT)}
                    evs = []
                    kt_outer = ch == 0 and bi == 0 and head_split
                    na_full = nal[bi]

                    seg = bi * PB
                    for mi in range(MT):
                        w = D - mi * 128
                        col0 = mi * 128
                        PR = psp.tile([128, w], F32, tag="ps",
                                      padded_shape=[128, D], name="pr")
                        PI = psp.tile([128, w], F32, tag="ps",
                                      padded_shape=[128, D], name="pi")
                        if "mm8" in parts:
                            # timing-shape probe: 12 fp8 DoubleRow matmuls
                            # (K=256 each) as the hi/lo-compensated scheme
                            # would issue. Operand values are garbage
                            # (bitcast of the fp16 chunk); timing-valid.
                            DR = mybir.MatmulPerfMode.DoubleRow
                            c8 = cht[:, boff:boff + IN_PB].bitcast(FP8)
                            n8 = na_full[:].bitcast(FP8)

                            def sl8(base, c0, n):
                                return base.rearrange(
                                    "p (k d) -> p k d", k=2)[:, :, c0:c0 + n]

                            sbases = [0, 512, 1024, 1536]  # ah, al, ch, cl
                            stats = [sl8(c8, sbases[j % 4] + col0, 128)
                                     if j % 3 else sl8(n8, (j % 2) * 512 + col0, 128)
                                     for j in range(12)]
                            movs = [sl8(c8, sbases[(j + 1) % 4] + col0, w)
                                    for j in range(12)]
                            banks = [PR, PI] * 6
                            for j in range(12):
                                nc.tensor.matmul(banks[j][:], stats[j], movs[j],
                                                 start=j < 2, stop=j >= 10,
                                                 perf_mode=DR)
                        elif "mm" in parts:
                            for kt in range(KT):
                                ak = boff + kt * 2 * D          # a cols base
                                ck = ak + D                      # c cols base
                                st_ = kt == 0
                                sp = kt == KT - 1
                                a_m = cht[:, ak + col0: ak + col0 + 128]
                                a_n = cht[:, ak + col0: ak + col0 + w]
                                c_m = cht[:, ck + col0: ck + col0 + 128]
                                c_n = cht[:, ck + col0: ck + col0 + w]
                                na_m = na_full[:, kt * D + col0: kt * D + col0 + 128]
                                mms[kt].append(
                                    (PR, a_m, a_n, st_, False))
                                mms[kt].append(
                                    (PI, c_m, a_n, st_, False))
                                mms[kt].append(
                                    (PI, na_m, c_n, False, sp))
                                mms[kt].append(
                                    (PR, c_m, c_n, False, sp))
                        # evictions: off-diag Ro on DVE, Io on ACT;
                        # diag: M = PR_d + PI_d (ACT copies PI_d to SBUF f32,
                        # DVE adds PSUM+SBUF -> fp16)
                        if "evict" in parts:
                            def ev(PR=PR, PI=PI, mi=mi, w=w, seg=seg):
                                if w > 128:
                                    nc.vector.tensor_copy(
                                        st[:, seg + RO_OFF[mi]: seg + RO_OFF[mi] + w - 128],
                                        PR[:, 128:w])
                                    nc.scalar.mul(
                                        st[:, seg + SEG_IO + RO_OFF[mi]:
                                           seg + SEG_IO + RO_OFF[mi] + w - 128],
                                        PI[:, 128:w], 1.0)
                                td = tmp.tile([128, 128], F32, tag="td", name=f"td{mi}")
                                nc.scalar.mul(td[:], PI[:, 0:128], 1.0)
                                nc.vector.tensor_add(
                                    st[:, seg + SEG_M + mi * 128: seg + SEG_M + (mi + 1) * 128],
                                    PR[:, 0:128], td[:])
                            evs.append(ev)

                    def issue(bank, l, r, st_, sp):
                        nc.tensor.matmul(bank[:], l, r, start=st_, stop=sp)

                    if kt_outer:
                        # a-only matmuls first (they need just the first
                        # 512-col load), then the rest of kt0, then kt1
                        first = [m for j, m in enumerate(mms[0]) if j % 4 == 0]
                        rest = [m for j, m in enumerate(mms[0]) if j % 4]
                        for m in first + rest + mms[1]:
                            issue(*m)
                        for ev in evs:
                            ev()
                    else:
                        for mi in range(MT):
                            for kt in range(KT):
                                for m in mms[kt][mi * 4:(mi + 1) * 4]:
                                    issue(*m)
                            if evs:
                                evs[mi]()
                if "store" in parts:
                    if ch == len(chunks) - 1 and tail_split:
                        # split the final store: Ro+Io (ready after mi2's
                        # eviction) early, the small M segment at the end
                        cut = (nb - 1) * PB + SEG_M
                        nc.gpsimd.dma_start(
                            outp[:, b0 * PB:b0 * PB + cut], st[:, :cut])
                        nc.gpsimd.dma_start(
                            outp[:, b0 * PB + cut:(b0 + nb) * PB],
                            st[:, cut:nb * PB])
                    else:
                        nc.gpsimd.dma_start(
                            outp[:, b0 * PB:(b0 + nb) * PB], st[:, :nb * PB])

        def warmup(n_mm=12):
            # PE p-state warmup: dummy matmuls (uninitialized SBUF operands,
            # discarded results) run while the first input DMA is in flight,
            # so real matmuls start at the full 2.4 GHz clock. Outside the
            # timing loop: steady-state cost is zero.
            wsrc = nap.tile([128, 2 * D], FP16, tag="wsrc")
            nc.vector.memset(wsrc[:], 0.0)
            wps = psp.tile([128, D], F32, tag="ps", padded_shape=[128, D],
                           name="wps")
            for j in range(n_mm):
                nc.tensor.matmul(wps[:], wsrc[:, 0:128], wsrc[:, D:2 * D],
                                 start=j == 0, stop=j == n_mm - 1)
            wo = nap.tile([128, 8], FP16, tag="wo")
            nc.vector.tensor_copy(wo[:], wps[:, 0:8])
            nc.gpsimd.dma_start(warm_out, wo[:])

        if "warm" not in parts:
            pass
        elif unroll <= 1:
            warmup()
        if unroll > 1:
            for _ in range(unroll):
                body()
        elif reps == 1:
            body()
        else:
            with tc.For_i(0, reps, 1) as iv:
                body(iv)

    nc.compile()
    return nc


_NC_CACHE = {}


def _get_nc(reps: int = 1):
    if reps not in _NC_CACHE:
        _NC_CACHE[reps] = build_nc(reps=reps)
    return _NC_CACHE[reps]


def make_in_maps(real, imag, weight):
    """Host prescale + pack: per core IN[128, BPC*IN_PB] fp16 where
    IN[p, ((b*KT + kt)*2 + role)*D + d] = fp16(sqrt(w[b,kt*128+p]) *
    {real,imag}[b, kt*128+p, d])."""
    sw = np.sqrt(weight.astype(np.float32))           # [B, T, 1]
    a = (sw * real).astype(np.float16).reshape(B_FULL, KT, 128, D)
    c = (sw * imag).astype(np.float16).reshape(B_FULL, KT, 128, D)
    ac = np.stack([a, c], axis=3)                     # [B, KT, 128, 2, D]
    maps = []
    for k in range(N_CORES):
        sub = ac[k * BPC:(k + 1) * BPC]               # [BPC, KT, 128, 2, D]
        x = np.ascontiguousarray(sub.transpose(2, 0, 1, 3, 4)
                                 ).reshape(128, BPC * IN_PB)
        maps.append({"inp": x})
    return maps


def _unpack(res_list):
    """Per-core outp [128, BPC*PB] fp16 -> full f32 (out_r, out_i)."""
    p = np.stack(res_list, axis=0).astype(np.float32)   # [NC, 128, BPC*PB]
    p = p.reshape(N_CORES, 128, BPC, PB).transpose(0, 2, 1, 3)
    p = p.reshape(B_FULL, 128, PB)                      # [B, 128, PB]
    out_r = np.empty((B_FULL, D, D), np.float32)
    out_i = np.empty((B_FULL, D, D), np.float32)
    for mi in range(MT):
        rs = slice(mi * 128, (mi + 1) * 128)
        # diag block: M = D_r + D_i
        M = p[:, :, SEG_M + mi * 128: SEG_M + (mi + 1) * 128]
        Mt = M.transpose(0, 2, 1)
        out_r[:, rs, rs] = (M + Mt) * 0.5
        out_i[:, rs, rs] = (M - Mt) * 0.5
        w = RO_W[mi]
        if w:
            cs = slice((mi + 1) * 128, D)
            out_r[:, rs, cs] = p[:, :, RO_OFF[mi]: RO_OFF[mi] + w]
            out_i[:, rs, cs] = p[:, :, SEG_IO + RO_OFF[mi]: SEG_IO + RO_OFF[mi] + w]
            # mirror lower blocks
            out_r[:, cs, rs] = out_r[:, rs, cs].transpose(0, 2, 1)
            out_i[:, cs, rs] = -out_i[:, rs, cs].transpose(0, 2, 1)
    return out_r, out_i


def kernel(real, imag, weight):
    real = np.asarray(real, dtype=np.float32)
    imag = np.asarray(imag, dtype=np.float32)
    weight = np.asarray(weight, dtype=np.float32)
    assert real.shape == (B_FULL, T, D) and weight.shape == (B_FULL, T, 1)

    nc = _get_nc()
    in_maps = make_in_maps(real, imag, weight)
    res = run_bass_kernel_spmd(nc, in_maps, list(range(N_CORES)))
    return _unpack([res.results[i]["outp"] for i in range(N_CORES)])


# revision 31
# speedup vs baseline: 1.6275x; 1.0064x over previous
"""Trainium2 Bass kernel for nn_ComplexMixture: weighted complex Gram matrices.

Reference (per batch b, inputs real/imag [B,T,D] f32, weight [B,T,1] f32):
    out_r[b] = sum_t w[b,t] * (r_t r_t^T + i_t i_t^T)   (symmetric)
    out_i[b] = sum_t w[b,t] * (i_t r_t^T - r_t i_t^T)   (antisymmetric)
with B=64, T=256, D=512; outputs (out_r, out_i), each [B, D, D] f32.

Pure data-parallel over 8 NeuronCores (8 batches per core). Final design:
  - Host pre-scales and packs: a = fp16(sqrt(w)*r), c = fp16(sqrt(w)*i),
    transposed to IN[128, (b,kt,role,d)] so every input DMA is fully
    contiguous per partition. Removes all on-device weight handling (v1
    spent ~1/3 of its time on a 2048x4B-descriptor weight DMA + ACT
    scaling) and halves input bytes. fp16 operands, exact f32 PSUM
    accumulation: rel err ~3.6e-4 vs the 2e-2 gate.
  - Only the upper block-trapezoid is computed (out_r symmetric, out_i
    antisymmetric; host mirrors the lower blocks): per (batch, mi row-
    block): PR = sum_kt a^T a + c^T c, PI = sum_kt c^T a + (-a)^T c,
    4 fp16 matmuls per (mi, kt) = 32 per batch. PE streaming is the
    roofline term: 81920 PE columns/core = 34.1 us at 2.4 GHz.
  - Diagonal-block combine: only M = D_r + D_i is stored for the 4
    diagonal blocks (host recovers D_r=(M+M^T)/2, D_i=(M-M^T)/2).
    Output 2048 fp16 cols/batch = 4.2 MB/core.
  - -a (for the -a^T c matmul) built on DVE (fp16 4x mode), loaded+built
    one chunk AHEAD of the matmuls (software pipeline) so the PE does not
    wait on the DVE queue behind the previous batch's evictions.
  - PSUM evictions split DVE / ACT; 8 single-bank PSUM tiles rotate so
    ~4 row-blocks are in flight across batch boundaries.
  - Loads: one 512KB DMA per batch, alternating the two HWDGE rings
    (sync/scalar). Measured: finer chunks beat coarser (50.7 vs 56/64 us
    for 2/4-batch chunks) - DMA fixed costs hide better and the na
    pipeline stays tight. First load split (a-kt0 | c-kt0 | rest) so
    batch 0's matmuls start ~2 us sooner (single-shot head). Stores per
    batch via SWDGE (gpsimd); last store split so the tail transfer
    after the final eviction is short.
Host: unpack fp16 -> f32, mirror lower blocks (r: +T, i: -T).

Measured (device-resident PJRT differencing, reps 1 vs 2049, median of
alternating rounds): ~50.8 us/iter vs ~143-172 us for the v1 baseline.
Engine budget at that point: PE ~34 us streaming + ~10 us of loop/sync
overhead (p-state ramps after small stalls), loads ~19 us and stores
~15 us largely hidden. Rejected after measurement: fp8e4 DoubleRow
matmuls (2x stream rate but 256-row weight loads do not overlap: 2.65x
SLOWER end-to-end); consolidated 6-op evictions + full PR/PI store
(v4: +1 MB output ate the op savings); PE warmup chain (sim: worse).
"""
import numpy as np
from contextlib import ExitStack

import concourse.bacc as bacc
import concourse.tile as tile
from concourse import mybir
from concourse.bass_utils import run_bass_kernel_spmd

F32 = mybir.dt.float32
FP16 = mybir.dt.float16
FP8 = mybir.dt.float8e4

N_CORES = 8
B_FULL = 64
BPC = B_FULL // N_CORES  # batches per core
T, D = 256, 512
KT = T // 128             # K tiles per batch
MT = D // 128              # output row blocks

# per-batch packed output layout (fp16 cols):
#   [Ro offdiag (768) | Io offdiag (768) | M diag blocks (512)] = 2048
RO_W = [D - 128 * (mi + 1) for mi in range(MT)]      # 384, 256, 128, 0
RO_OFF = [sum(RO_W[:j]) for j in range(MT)]          # 0, 384, 640, 768
SEG_IO = sum(RO_W)                                   # 768
SEG_M = 2 * SEG_IO                                   # 1536
PB = SEG_M + MT * 128                                # 2048 cols per batch
IN_PB = KT * 2 * D                                   # 2048 input cols per batch
CHB = 2                                              # batches per DMA chunk


ALL_PARTS = frozenset({"load", "neg", "mm", "evict", "store"})


def build_nc(reps: int = 1, unroll: int = 1,
             ld_bufs=3, na_bufs=4, st_bufs=3, tmp_bufs=8, ps_bufs=8,
             parts=ALL_PARTS, chunks=(1,) * BPC,
             head_split=True, tail_split=True, store_hwdge=True):
    """Build + compile the per-core program. reps>1 wraps the body in a
    hardware loop (timing only; output idempotent). unroll>1 python-unrolls
    (for the timeline simulator, which can't run For_i). parts: ablation
    subsets for bench.py (timing experiments only)."""
    nc = bacc.Bacc("TRN2", target_bir_lowering=False, debug=False)
    inp = nc.dram_tensor("inp", [128, BPC * IN_PB], FP16,
                         kind="ExternalInput").ap()
    outp = nc.dram_tensor("outp", [128, BPC * PB], FP16,
                          kind="ExternalOutput").ap()
    warm_out = nc.dram_tensor("warm", [128, 8], FP16,
                              kind="ExternalOutput").ap()

    with tile.TileContext(nc) as tc, ExitStack() as ctx:
        ld = ctx.enter_context(tc.tile_pool(name="ld", bufs=ld_bufs))
        nap = ctx.enter_context(tc.tile_pool(name="nap", bufs=na_bufs))
        stp = ctx.enter_context(tc.tile_pool(name="stp", bufs=st_bufs))
        tmp = ctx.enter_context(tc.tile_pool(name="tmp", bufs=tmp_bufs))
        psp = ctx.enter_context(tc.tile_pool(name="psp", bufs=ps_bufs, space="PSUM"))

        def body(_iv=None):
            assert sum(chunks) == BPC
            nbmax = max(chunks)
            b0s = [sum(chunks[:j]) for j in range(len(chunks))]

            def load_chunk(ch, b0, nb):
                # load + build this chunk's -a tiles; called one chunk
                # ahead of the matmuls so the PE never waits on the DVE
                # queue behind older evictions (priority inversion)
                cht = ld.tile([128, nbmax * IN_PB], FP16, tag="ch",
                              name=f"cht{ch}")
                if "load" in parts:
                    eng = nc.sync if ch % 2 == 0 else nc.scalar
                    eng2 = nc.scalar if ch % 2 == 0 else nc.sync
                    if ch == 0 and head_split:
                        # split the first load (a-kt0 | c-kt0 | rest) so
                        # batch 0's first matmuls start ~1.7 us sooner
                        i0 = b0 * IN_PB
                        eng.dma_start(cht[:, :D], inp[:, i0:i0 + D])
                        eng2.dma_start(cht[:, D:2 * D], inp[:, i0 + D:i0 + 2 * D])
                        eng.dma_start(cht[:, 2 * D:nb * IN_PB],
                                      inp[:, i0 + 2 * D:i0 + nb * IN_PB])
                    else:
                        eng.dma_start(cht[:, :nb * IN_PB],
                                      inp[:, b0 * IN_PB:(b0 + nb) * IN_PB])
                nal = []
                for bi in range(nb):
                    na_full = nap.tile([128, KT * D], FP16, tag="naf",
                                       name=f"na{ch}_{bi}")
                    if "neg" in parts:
                        boff = bi * IN_PB
                        for kt in range(KT):
                            nc.vector.tensor_scalar_mul(
                                na_full[:, kt * D:(kt + 1) * D],
                                cht[:, boff + kt * 2 * D: boff + kt * 2 * D + D],
                                -1.0)
                    nal.append(na_full)
                return cht, nal

            chts = {0: load_chunk(0, b0s[0], chunks[0])}
            for ch, (b0, nb) in enumerate(zip(b0s, chunks)):
                cht, nal = chts.pop(ch)
                if ch + 1 < len(chunks):
                    chts[ch + 1] = load_chunk(ch + 1, b0s[ch + 1], chunks[ch + 1])

                st = stp.tile([128, nbmax * PB], FP16, tag="st")
                for bi in range(nb):
                    boff = bi * IN_PB
                    mms = {kt: [] for kt in range(KT)}
                    evs = []
                    kt_outer = ch == 0 and bi == 0 and head_split
                    na_full = nal[bi]

                    seg = bi * PB
                    for mi in range(MT):
                        w = D - mi * 128
                        col0 = mi * 128
                        PR = psp.tile([128, w], F32, tag="ps",
                                      padded_shape=[128, D], name="pr")
                        PI = psp.tile([128, w], F32, tag="ps",
                                      padded_shape=[128, D], name="pi")
                        if "mm8" in parts:
                            # timing-shape probe: 12 fp8 DoubleRow matmuls
                            # (K=256 each) as the hi/lo-compensated scheme
                            # would issue. Operand values are garbage
                            # (bitcast of the fp16 chunk); timing-valid.
                            DR = mybir.MatmulPerfMode.DoubleRow
                            c8 = cht[:, boff:boff + IN_PB].bitcast(FP8)
                            n8 = na_full[:].bitcast(FP8)

                            def sl8(base, c0, n):
                                return base.rearrange(
                                    "p (k d) -> p k d", k=2)[:, :, c0:c0 + n]

                            sbases = [0, 512, 1024, 1536]  # ah, al, ch, cl
                            stats = [sl8(c8, sbases[j % 4] + col0, 128)
                                     if j % 3 else sl8(n8, (j % 2) * 512 + col0, 128)
                                     for j in range(12)]
                            movs = [sl8(c8, sbases[(j + 1) % 4] + col0, w)
                                    for j in range(12)]
                            banks = [PR, PI] * 6
                            for j in range(12):
                                nc.tensor.matmul(banks[j][:], stats[j], movs[j],
                                                 start=j < 2, stop=j >= 10,
                                                 perf_mode=DR)
                        elif "mm" in parts:
                            for kt in range(KT):
                                ak = boff + kt * 2 * D          # a cols base
                                ck = ak + D                      # c cols base
                                st_ = kt == 0
                                sp = kt == KT - 1
                                a_m = cht[:, ak + col0: ak + col0 + 128]
                                a_n = cht[:, ak + col0: ak + col0 + w]
                                c_m = cht[:, ck + col0: ck + col0 + 128]
                                c_n = cht[:, ck + col0: ck + col0 + w]
                                na_m = na_full[:, kt * D + col0: kt * D + col0 + 128]
                                mms[kt].append(
                                    (PR, a_m, a_n, st_, False))
                                mms[kt].append(
                                    (PI, c_m, a_n, st_, False))
                                mms[kt].append(
                                    (PI, na_m, c_n, False, sp))
                                mms[kt].append(
                                    (PR, c_m, c_n, False, sp))
                        # evictions: off-diag Ro on DVE, Io on ACT;
                        # diag: M = PR_d + PI_d (ACT copies PI_d to SBUF f32,
                        # DVE adds PSUM+SBUF -> fp16)
                        if "evict" in parts:
                            def ev(PR=PR, PI=PI, mi=mi, w=w, seg=seg):
                                if w > 128:
                                    nc.vector.tensor_copy(
                                        st[:, seg + RO_OFF[mi]: seg + RO_OFF[mi] + w - 128],
                                        PR[:, 128:w])
                                    nc.scalar.mul(
                                        st[:, seg + SEG_IO + RO_OFF[mi]:
                                           seg + SEG_IO + RO_OFF[mi] + w - 128],
                                        PI[:, 128:w], 1.0)
                                td = tmp.tile([128, 128], F32, tag="td", name=f"td{mi}")
                                nc.scalar.mul(td[:], PI[:, 0:128], 1.0)
                                nc.vector.tensor_add(
                                    st[:, seg + SEG_M + mi * 128: seg + SEG_M + (mi + 1) * 128],
                                    PR[:, 0:128], td[:])
                            evs.append(ev)

                    def issue(bank, l, r, st_, sp):
                        nc.tensor.matmul(bank[:], l, r, start=st_, stop=sp)

                    if kt_outer:
                        # a-only matmuls first (they need just the first
                        # 512-col load), then the rest of kt0, then kt1
                        first = [m for j, m in enumerate(mms[0]) if j % 4 == 0]
                        rest = [m for j, m in enumerate(mms[0]) if j % 4]
                        for m in first + rest + mms[1]:
                            issue(*m)
                        for ev in evs:
                            ev()
                    else:
                        for mi in range(MT):
                            for kt in range(KT):
                                for m in mms[kt][mi * 4:(mi + 1) * 4]:
                                    issue(*m)
                            if evs:
                                evs[mi]()
                if "store" in parts:
                    # opposite HWDGE ring to this chunk's load, or SWDGE
                    seng = (nc.scalar if ch % 2 == 0 else nc.sync) \
                        if store_hwdge else nc.gpsimd
                    if ch == len(chunks) - 1 and tail_split:
                        # split the final store: Ro+Io (ready after mi2's
                        # eviction) early, the small M segment at the end
                        cut = (nb - 1) * PB + SEG_M
                        seng.dma_start(
                            outp[:, b0 * PB:b0 * PB + cut], st[:, :cut])
                        seng.dma_start(
                            outp[:, b0 * PB + cut:(b0 + nb) * PB],
                            st[:, cut:nb * PB])
                    else:
                        seng.dma_start(
                            outp[:, b0 * PB:(b0 + nb) * PB], st[:, :nb * PB])

        def warmup(n_mm=12):
            # PE p-state warmup: dummy matmuls (uninitialized SBUF operands,
            # discarded results) run while the first input DMA is in flight,
            # so real matmuls start at the full 2.4 GHz clock. Outside the
            # timing loop: steady-state cost is zero.
            wsrc = nap.tile([128, 2 * D], FP16, tag="wsrc")
            nc.vector.memset(wsrc[:], 0.0)
            wps = psp.tile([128, D], F32, tag="ps", padded_shape=[128, D],
                           name="wps")
            for j in range(n_mm):
                nc.tensor.matmul(wps[:], wsrc[:, 0:128], wsrc[:, D:2 * D],
                                 start=j == 0, stop=j == n_mm - 1)
            wo = nap.tile([128, 8], FP16, tag="wo")
            nc.vector.tensor_copy(wo[:], wps[:, 0:8])
            nc.gpsimd.dma_start(warm_out, wo[:])

        if "warm" not in parts:
            pass
        elif unroll <= 1:
            warmup()
        if unroll > 1:
            for _ in range(unroll):
                body()
        elif reps == 1:
            body()
        else:
            with tc.For_i(0, reps, 1) as iv:
                body(iv)

    nc.compile()
    return nc


_NC_CACHE = {}


def _get_nc(reps: int = 1):
    if reps not in _NC_CACHE:
        _NC_CACHE[reps] = build_nc(reps=reps)
    return _NC_CACHE[reps]


def make_in_maps(real, imag, weight):
    """Host prescale + pack: per core IN[128, BPC*IN_PB] fp16 where
    IN[p, ((b*KT + kt)*2 + role)*D + d] = fp16(sqrt(w[b,kt*128+p]) *
    {real,imag}[b, kt*128+p, d])."""
    sw = np.sqrt(weight.astype(np.float32))           # [B, T, 1]
    a = (sw * real).astype(np.float16).reshape(B_FULL, KT, 128, D)
    c = (sw * imag).astype(np.float16).reshape(B_FULL, KT, 128, D)
    ac = np.stack([a, c], axis=3)                     # [B, KT, 128, 2, D]
    maps = []
    for k in range(N_CORES):
        sub = ac[k * BPC:(k + 1) * BPC]               # [BPC, KT, 128, 2, D]
        x = np.ascontiguousarray(sub.transpose(2, 0, 1, 3, 4)
                                 ).reshape(128, BPC * IN_PB)
        maps.append({"inp": x})
    return maps


def _unpack(res_list):
    """Per-core outp [128, BPC*PB] fp16 -> full f32 (out_r, out_i)."""
    p = np.stack(res_list, axis=0).astype(np.float32)   # [NC, 128, BPC*PB]
    p = p.reshape(N_CORES, 128, BPC, PB).transpose(0, 2, 1, 3)
    p = p.reshape(B_FULL, 128, PB)                      # [B, 128, PB]
    out_r = np.empty((B_FULL, D, D), np.float32)
    out_i = np.empty((B_FULL, D, D), np.float32)
    for mi in range(MT):
        rs = slice(mi * 128, (mi + 1) * 128)
        # diag block: M = D_r + D_i
        M = p[:, :, SEG_M + mi * 128: SEG_M + (mi + 1) * 128]
        Mt = M.transpose(0, 2, 1)
        out_r[:, rs, rs] = (M + Mt) * 0.5
        out_i[:, rs, rs] = (M - Mt) * 0.5
        w = RO_W[mi]
        if w:
            cs = slice((mi + 1) * 128, D)
            out_r[:, rs, cs] = p[:, :, RO_OFF[mi]: RO_OFF[mi] + w]
            out_i[:, rs, cs] = p[:, :, SEG_IO + RO_OFF[mi]: SEG_IO + RO_OFF[mi] + w]
            # mirror lower blocks
            out_r[:, cs, rs] = out_r[:, rs, cs].transpose(0, 2, 1)
            out_i[:, cs, rs] = -out_i[:, rs, cs].transpose(0, 2, 1)
    return out_r, out_i


def kernel(real, imag, weight):
    real = np.asarray(real, dtype=np.float32)
    imag = np.asarray(imag, dtype=np.float32)
    weight = np.asarray(weight, dtype=np.float32)
    assert real.shape == (B_FULL, T, D) and weight.shape == (B_FULL, T, 1)

    nc = _get_nc()
    in_maps = make_in_maps(real, imag, weight)
    res = run_bass_kernel_spmd(nc, in_maps, list(range(N_CORES)))
    return _unpack([res.results[i]["outp"] for i in range(N_CORES)])
